# revision 25
# baseline (speedup 1.0000x reference)
"""Trainium2 Bass kernel for windowed (local) causal self-attention.

Reference computation (per batch element, fp32):
    q = x @ Wq.T + bq ; k = x @ Wk.T + bk ; v = x @ Wv.T + bv
    per non-overlapping window of 256 tokens:
        attn = softmax(causal_mask(q k^T * HEAD_DIM**-0.5))
        out  = attn @ v
    o = out @ Wo.T + bo + x

Algebraic reduction (the reference has no head split, so scores contract
over the full E=1024):
    q_i.k_j = x_i^T (Wq^T Wk) x_j + x_i.(Wq^T bk) + (Wk^T bq).x_j + bq.bk
The second and fourth terms are constant along the softmax axis and
cancel; with M = Wq^T Wk and vvec = Wk^T bq:
    scores = (X M + 1 vvec^T) X^T      (one projection instead of two)
Since softmax rows sum to 1,
    attn @ (X Wv^T + 1 bv^T) @ Wo^T + 1 bo^T = attn @ (X P^T) + 1 b'^T
with P = Wo Wv and b' = Wo bv + bo (one projection instead of two).
b' is folded into the residual copy of x on the host.  Device work per
token is 2 E^2 projection MACs + windowed attention: ~55% of the naive
PE work, which puts the kernel at the PE roofline (~131us of 1c/row
matmul at 2.4GHz per core).

Sharding: data-parallel over (batch, window): 64 window-blocks of 256
tokens -> 8 cores x 8 windows.  M/P replicated.

Per-core kernel strategy (PE matmul runs 1 cycle/row for fp16 and
fp32r alike, so fp16 is used where it halves DMA, f32r where range
matters):
  - M, P resident in SBUF as fp16 (4MB); x streamed as host-transposed
    fp16 xT plus a token-major fp16 residual copy (x + b').
  - windows processed in groups [(0,),(1,),(2,3),(4,5),(6,7)]: the solo
    groups let compute start after ~1.5MB of DMA (first M half + a
    host-packed xT tile with 4KB DMA rows); the pairs run the
    y-projection at N=512 to halve the stationary-load count.  Weight
    chunks are ordered by compute deadline and paced with tiny PE
    warmup transposes to keep the clock ramped.
  - scores are computed TRANSPOSED: scT[k,q] = sum_e xT[e,k] yT[e,q],
    reusing yT as the moving operand (no kT).  The causal mask (also
    transposed) is added on DVE and exp (scale fused) runs on ACT in
    fp32r - no max-subtraction needed at fp32 range, and no PE
    transposes of the attention matrix at all.
  - softmax normalization is deferred: row sums come from a tiny
    ones-column matmul over the k partitions (psum [q,1]), 1/sum on
    DVE, and the scale is fused into the output evacuation as one DVE
    scalar_tensor_tensor: o = (po * rec) + resid -> fp16 store.
  - Z = X P^T token-major in fp16 psum->f32r tiles (ACT-copy evac);
    out = exp_T^T @ Z accumulated per (qt, eoh) with the causally-zero
    (qt0, ktt1) block skipped entirely.
  - queues: sync carries weights/xT/xres prefetches and eoh0 stores;
    gpsimd carries constants and eoh1 stores; ACT stays on the softmax
    critical path; output stores never queue behind prefetches.
"""
import sys

sys.path.insert(0, "/opt/trn_rl_repo")

import numpy as np

import concourse.bass as bass
import concourse.bacc as bacc
import concourse.mybir as mybir
import concourse.tile as tile
from concourse.bass_utils import run_bass_kernel_spmd

F32 = mybir.dt.float32
F32R = mybir.dt.float32r
F16 = mybir.dt.float16
AF = mybir.ActivationFunctionType

E = 1024          # embed dim
ET = E // 128     # e-tiles
W = 256           # window size
NW = 8            # windows per core
T = NW * W        # tokens per core
N_CORES = 8
SCALE = (E // 16) ** (-0.5)  # HEAD_DIM ** -0.5 = 0.125
NEG = -1.0e30

GROUPS = [(0,), (1,), (2, 3), (4, 5), (6, 7)]


def build_nc(nw=NW):
    t_core = nw * W
    nc = bacc.Bacc("TRN2", target_bir_lowering=False, debug=False)

    # x: token-major residual copy with b' = Wo@bv + bo pre-added (host)
    x_d = nc.dram_tensor("x", [t_core, E], F16, kind="ExternalInput")
    xt_d = nc.dram_tensor("xt", [E, t_core], F16, kind="ExternalInput")
    # windows 0/1 pre-packed on the host in SBUF tile layout -> 4KB DMA rows
    xs_d = [
        nc.dram_tensor(f"xs{i}", [128, ET * W], F16, kind="ExternalInput")
        for i in range(2)
    ]
    m_d = nc.dram_tensor("wm", [E, E], F16, kind="ExternalInput")   # Wq^T @ Wk
    p_d = nc.dram_tensor("wz", [E, E], F16, kind="ExternalInput")   # (Wo @ Wv)^T
    vv_d = nc.dram_tensor("vv", [128, ET], F32, kind="ExternalInput")  # Wk^T @ bq
    o_d = nc.dram_tensor("o", [t_core, E], F16, kind="ExternalOutput")

    # host-side constants baked into the NEFF
    # transposed-layout causal masks: maskT[ktt][k, q] = NEG where q < k
    # (scores are computed as scT[k, q]; softmax runs along the k partitions)
    mask_np = np.zeros((2, 128, W), dtype=np.float32)
    k_i = np.arange(128)[:, None]
    q_i = np.arange(W)[None, :]
    mask_np[0][q_i < k_i] = NEG
    mask_np[1][:, :128][q_i[:, :128] < k_i] = NEG
    mask_d = nc.inline_tensor(mask_np, "mask")
    ident_d = nc.inline_tensor(np.eye(128, dtype=np.float16), "ident")
    ones_d = nc.inline_tensor(np.ones((128, 4), dtype=np.float32), "ones")

    with tile.TileContext(nc) as tc:
        with (
            tc.tile_pool(name="wgt", bufs=1) as wp,
            tc.tile_pool(name="cp", bufs=1) as cp,
            tc.tile_pool(name="xp", bufs=4) as xp,
            tc.tile_pool(name="xtp", bufs=2) as xtp,
            tc.tile_pool(name="ytp", bufs=2) as ytp,
            tc.tile_pool(name="zp", bufs=2) as zp,
            tc.tile_pool(name="sp", bufs=2) as sp,
            tc.tile_pool(name="smp", bufs=4) as smp,
            tc.tile_pool(name="op", bufs=3) as op,
            tc.tile_pool(name="ps_y", bufs=2, space=bass.MemorySpace.PSUM) as ps_y,
            tc.tile_pool(name="ps_z", bufs=2, space=bass.MemorySpace.PSUM) as ps_z,
            tc.tile_pool(name="ps_o", bufs=3, space=bass.MemorySpace.PSUM) as ps_o,
            tc.tile_pool(name="ps_tr", bufs=1, space=bass.MemorySpace.PSUM) as ps_tr,
        ):
            # ---- resident constants (gpsimd queue; tiny) ----
            ident = cp.tile([128, 128], F16, tag="ident")
            nc.gpsimd.dma_start(ident[:], ident_d.ap())
            masks = cp.tile([128, 2, W], F32, tag="mask")
            for qt in range(2):
                nc.gpsimd.dma_start(masks[:, qt, :], mask_d.ap()[qt])
            vv_sb = cp.tile([128, ET], F32, tag="vv")
            nc.gpsimd.dma_start(vv_sb[:], vv_d.ap())
            ones_col = cp.tile([128, 4], F32R, tag="ones")
            nc.gpsimd.dma_start(ones_col[:], ones_d.ap().bitcast(F32R))

            # ---- resident weights: wsb[m][p, ei, eo] = Wm[ei*128+p, eo] ----
            wsb = {
                "m": wp.tile([128, ET, E], F16, tag="wm", name="wmsb"),
                "z": wp.tile([128, ET, E], F16, tag="wz", name="wzsb"),
            }
            w_d = {"m": m_d, "z": p_d}

            def load_weight(m, half, warm=False, eq_list=(0, 4)):
                # one 3D DMA per (half, ei-quadrant): 0.5MB transfers with
                # 1KB contiguous rows keep the DMA engines near full rate
                wr = w_d[m].ap().rearrange("(a p) n -> a p n", p=128)
                for eq in eq_list:
                    nc.sync.dma_start(
                        wsb[m][:, eq : eq + 4, half * 512 : (half + 1) * 512],
                        wr[eq : eq + 4, :, half * 512 : (half + 1) * 512].transpose(
                            [1, 0, 2]
                        ),
                    )
                    if warm:
                        # keep the PE activity monitor warm through the
                        # DMA-bound phase: a tiny transpose per arriving
                        # chunk, paced by the DMA itself
                        wps = ps_tr.tile([128, 128], F16, tag="tr", name="warm")
                        nc.tensor.transpose(
                            wps[:],
                            wsb[m][:, eq, half * 512 : half * 512 + 128],
                            ident[:],
                        )

            xtr = xt_d.ap().rearrange("(a p) t -> a p t", p=128)
            xT_tiles = {}

            def load_xT(gi, parts=(0, 1)):
                g = GROUPS[gi]
                if len(g) == 1:
                    # solo startup windows: host-packed tile layout loads as
                    # linear DMAs with 4KB rows; split by ei-half so the
                    # first y matmuls (ei 0-3) start one chunk earlier
                    if gi not in xT_tiles:
                        xT_tiles[gi] = xtp.tile([128, ET, W], F16, tag="xT0", name=f"xT0_{gi}")
                    t = xT_tiles[gi]
                    srcr = xs_d[g[0]].ap().rearrange("p (a t) -> p a t", a=ET)
                    for pt in parts:
                        nc.sync.dma_start(
                            t[:, pt * 4 : (pt + 1) * 4, :],
                            srcr[:, pt * 4 : (pt + 1) * 4, :],
                        )
                    return
                else:
                    t = xtp.tile([128, ET, 2 * W], F16, tag="xT")
                    nc.sync.dma_start(
                        t[:, :, :],
                        xtr[:, :, g[0] * W : (g[0] + 2) * W].transpose([1, 0, 2]),
                    )
                xT_tiles[gi] = t

            for gi, g in enumerate(GROUPS):
                pw = len(g) * W

                if gi == 0:
                    # sync-queue order = deadline order for window 0/1:
                    # M-h0 + xs0 gate the first y matmuls; M-h1 by y-eo4
                    # (~+3us); P halves by the w0 Z-projection (~+8/10us);
                    # xs1 only by w1's y (~+16us)
                    load_weight("m", half=0, warm=True, eq_list=(0,))
                    load_xT(0, parts=(0,))
                    load_weight("m", half=0, warm=True, eq_list=(4,))
                    load_xT(0, parts=(1,))
                    load_weight("m", half=1, warm=True)
                    load_weight("z", half=0, warm=True)
                    load_weight("z", half=1, warm=True)
                    load_xT(1)  # prefetch window 1 behind the weights
                elif gi + 1 < len(GROUPS):
                    load_xT(gi + 1)
                xT = xT_tiles[gi]

                # ---- y projection -> [e_out, t] layout, vvec bias fused ----
                yT = ytp.tile([128, ET, 2 * W], F16, tag="yT")
                for eo in range(ET):
                    pp = ps_y.tile([128, 2 * W], F32, tag="y")
                    for ei in range(ET):
                        nc.tensor.matmul(
                            pp[:, :pw],
                            wsb["m"][:, ei, eo * 128 : (eo + 1) * 128],
                            xT[:, ei, :pw],
                            start=(ei == 0),
                            stop=(ei == ET - 1),
                        )
                    if eo % 2 == 0:
                        nc.scalar.add(
                            yT[:, eo, :pw], pp[:, :pw], vv_sb[:, eo : eo + 1]
                        )
                    else:
                        nc.vector.tensor_scalar_add(
                            yT[:, eo, :pw], pp[:, :pw], vv_sb[:, eo : eo + 1]
                        )


                for wi, w in enumerate(g):
                    tok0 = w * W
                    two0 = wi * W  # token offset inside the group tiles

                    # residual (x + b'), token-major; needed only at out evac
                    x_w = []
                    for tt in range(2):
                        xt_ = xp.tile([128, E], F16, tag="x")
                        nc.sync.dma_start(
                            xt_[:],
                            x_d.ap()[tok0 + tt * 128 : tok0 + (tt + 1) * 128, :],
                        )
                        x_w.append(xt_)

                    # ---- transposed scores scT[k, q] + unnormalized exp ----
                    # softmax normalization is deferred: row sums come from a
                    # ones-column matmul over the k partitions and 1/sum is
                    # fused into the output evacuation.  exp stays fp32 so no
                    # max-subtraction is needed (f32 range covers e^30).
                    ex = []
                    for ktt in range(2):
                        qlo = ktt * 128  # causal: k-block ktt only sees q >= qlo
                        qw = W - qlo
                        sc = ps_z.tile([128, 512], F32, tag="z")
                        for ei in range(ET):
                            nc.tensor.matmul(
                                sc[:, :qw],
                                xT[:, ei, two0 + ktt * 128 : two0 + (ktt + 1) * 128],
                                yT[:, ei, two0 + qlo : two0 + W],
                                start=(ei == 0),
                                stop=(ei == ET - 1),
                            )
                        e_sb = sp.tile([128, W], F32R, tag="s", name=f"e{ktt}")
                        nc.vector.tensor_add(
                            e_sb[:, :qw], sc[:, :qw], masks[:, ktt, :qw]
                        )
                        nc.scalar.activation(
                            e_sb[:, :qw], e_sb[:, :qw], AF.Exp, scale=SCALE
                        )
                        ex.append(e_sb)


                    # ---- Z projection (token-major): Z = X P^T ----
                    z_w = [
                        zp.tile([128, E], F32R, tag="z", name=f"z{tt}")
                        for tt in range(2)
                    ]
                    for eoh in range(2):
                        for tt in range(2):
                            pv = ps_z.tile([128, 512], F32, tag="z")
                            for ei in range(ET):
                                nc.tensor.matmul(
                                    pv[:],
                                    xT[:, ei, two0 + tt * 128 : two0 + (tt + 1) * 128],
                                    wsb["z"][:, ei, eoh * 512 : (eoh + 1) * 512],
                                    start=(ei == 0),
                                    stop=(ei == ET - 1),
                                )
                            nc.scalar.copy(
                                z_w[tt][:, eoh * 512 : (eoh + 1) * 512], pv[:]
                            )

                    # ---- row sums (over k partitions) and reciprocals ----
                    recs = []
                    for qt in range(2):
                        # exp block for query column range qt: ex[0][:, qt*128:...]
                        # plus (qt1 only) ex[1][:, 0:128]
                        srcs = [(0, 0)] if qt == 0 else [(0, 128), (1, 0)]
                        sm = ps_o.tile([128, 512], F32, tag="o")
                        for i, (ktt, qo) in enumerate(srcs):
                            nc.tensor.matmul(
                                sm[:, :4],
                                ex[ktt][:, qo : qo + 128],
                                ones_col[:],
                                start=(i == 0),
                                stop=(i == len(srcs) - 1),
                            )
                        rec = smp.tile([128, 1], F32, tag="rec")
                        nc.vector.reciprocal(rec[:], sm[:, :1])
                        recs.append(rec)

                    # ---- out = (expT^T @ Z) * rec + residual, token-major ----
                    for qt in range(2):
                        srcs = [(0, 0)] if qt == 0 else [(0, 128), (1, 0)]
                        for eoh in range(2):
                            po = ps_o.tile([128, 512], F32, tag="o")
                            for i, (ktt, qo) in enumerate(srcs):
                                nc.tensor.matmul(
                                    po[:],
                                    ex[ktt][:, qo : qo + 128],
                                    z_w[ktt][:, eoh * 512 : (eoh + 1) * 512],
                                    start=(i == 0),
                                    stop=(i == len(srcs) - 1),
                                )
                            o_sb = op.tile([128, 512], F16, tag="o")
                            if w == nw - 1 and eoh == 1:
                                # tail drain: scale on ACT (per-partition
                                # scale AP), residual add on GpSimd (SBUF
                                # only) so the last four evacs run on three
                                # engines in parallel instead of serially
                                # on DVE
                                o_t = op.tile([128, 512], F32, tag="ot")
                                nc.scalar.activation(
                                    o_t[:], po[:], AF.Copy, scale=recs[qt][:]
                                )
                                nc.gpsimd.tensor_add(
                                    o_sb[:],
                                    o_t[:],
                                    x_w[qt][:, eoh * 512 : (eoh + 1) * 512],
                                )
                            else:
                                nc.vector.scalar_tensor_tensor(
                                    o_sb[:],
                                    po[:],
                                    recs[qt][:],
                                    x_w[qt][:, eoh * 512 : (eoh + 1) * 512],
                                    op0=mybir.AluOpType.mult,
                                    op1=mybir.AluOpType.add,
                                )
                            # stores on the sync/gpsimd queues (idle in
                            # steady state): the scalar queue's ACT ops sit
                            # on the softmax critical path, keep it clear
                            if w == nw - 1:
                                q = (nc.sync, nc.scalar, nc.sync, nc.scalar)[
                                    qt * 2 + eoh
                                ]
                            else:
                                q = nc.sync if eoh == 0 else nc.gpsimd
                            q.dma_start(
                                o_d.ap()[
                                    tok0 + qt * 128 : tok0 + (qt + 1) * 128,
                                    eoh * 512 : (eoh + 1) * 512,
                                ],
                                o_sb[:],
                                single_packet=(w == nw - 1),
                            )

    nc.compile()
    return nc


_NC_CACHE = {}


def _get_nc(nw=NW):
    if nw not in _NC_CACHE:
        _NC_CACHE[nw] = build_nc(nw)
    return _NC_CACHE[nw]


def _prep(x, Wq, bq, Wk, bk, Wv, bv, Wo, bo):
    """Host-side weight folding + per-core input maps."""
    x = np.asarray(x, dtype=np.float32)
    B, S, _ = x.shape
    Wq = np.asarray(Wq, np.float32)
    Wk = np.asarray(Wk, np.float32)
    Wv = np.asarray(Wv, np.float32)
    Wo = np.asarray(Wo, np.float32)
    bq = np.asarray(bq, np.float32)
    bv = np.asarray(bv, np.float32)
    bo = np.asarray(bo, np.float32)

    M = Wq.T @ Wk                      # scores = (X M) X^T  (+ col bias)
    Pt = (Wo @ Wv).T                   # Z = X @ Pt
    vvec = Wk.T @ bq                   # col bias, fused into y-projection
    bprime = Wo @ bv + bo              # folded into the residual below

    x_flat = x.reshape(B * S, E)
    x_resid = x_flat + bprime[None, :]
    t_core = B * S // N_CORES
    assert t_core == T

    common = {
        "wm": np.ascontiguousarray(M.astype(np.float16)),
        "wz": np.ascontiguousarray(Pt.astype(np.float16)),
        "vv": np.ascontiguousarray(vvec.reshape(ET, 128).T),
    }
    in_maps = []
    for i in range(N_CORES):
        xt_i = x_flat[i * t_core : (i + 1) * t_core].T.astype(np.float16)
        m = {
            "x": np.ascontiguousarray(
                x_resid[i * t_core : (i + 1) * t_core].astype(np.float16)
            ),
            "xt": np.ascontiguousarray(xt_i),
            **common,
        }
        for w in range(2):
            sl = xt_i[:, w * W : (w + 1) * W]  # [E, W]
            m[f"xs{w}"] = np.ascontiguousarray(
                sl.reshape(ET, 128, W).transpose(1, 0, 2).reshape(128, ET * W)
            )
        in_maps.append(m)
    return in_maps


def kernel(x, Wq, bq, Wk, bk, Wv, bv, Wo, bo):
    in_maps = _prep(x, Wq, bq, Wk, bk, Wv, bv, Wo, bo)
    B, S = np.asarray(x).shape[:2]
    nc = _get_nc()
    res = run_bass_kernel_spmd(nc, in_maps, core_ids=list(range(N_CORES)))
    out = np.concatenate([res.results[i]["o"] for i in range(N_CORES)], axis=0)
    return out.reshape(B, S, E).astype(np.float32)


# revision 26
# speedup vs baseline: 1.0126x; 1.0126x over previous
"""Trainium2 Bass kernel for windowed (local) causal self-attention.

Reference computation (per batch element, fp32):
    q = x @ Wq.T + bq ; k = x @ Wk.T + bk ; v = x @ Wv.T + bv
    per non-overlapping window of 256 tokens:
        attn = softmax(causal_mask(q k^T * HEAD_DIM**-0.5))
        out  = attn @ v
    o = out @ Wo.T + bo + x

Algebraic reduction (the reference has no head split, so scores contract
over the full E=1024):
    q_i.k_j = x_i^T (Wq^T Wk) x_j + x_i.(Wq^T bk) + (Wk^T bq).x_j + bq.bk
The second and fourth terms are constant along the softmax axis and
cancel; with M = Wq^T Wk and vvec = Wk^T bq:
    scores = (X M + 1 vvec^T) X^T      (one projection instead of two)
Since softmax rows sum to 1,
    attn @ (X Wv^T + 1 bv^T) @ Wo^T + 1 bo^T = attn @ (X P^T) + 1 b'^T
with P = Wo Wv and b' = Wo bv + bo (one projection instead of two).
b' is folded into the residual copy of x on the host.  Device work per
token is 2 E^2 projection MACs + windowed attention: ~55% of the naive
PE work, which puts the kernel at the PE roofline (~131us of 1c/row
matmul at 2.4GHz per core).

Sharding: data-parallel over (batch, window): 64 window-blocks of 256
tokens -> 8 cores x 8 windows.  M/P replicated.

Per-core kernel strategy (PE matmul runs 1 cycle/row for fp16 and
fp32r alike, so fp16 is used where it halves DMA, f32r where range
matters):
  - M, P resident in SBUF as fp16 (4MB); x streamed as host-transposed
    fp16 xT plus a token-major fp16 residual copy (x + b').
  - windows processed in groups [(0,),(1,),(2,3),(4,5),(6,7)]: the solo
    groups let compute start after ~1.5MB of DMA (first M half + a
    host-packed xT tile with 4KB DMA rows); the pairs run the
    y-projection at N=512 to halve the stationary-load count.  Weight
    chunks are ordered by compute deadline and paced with tiny PE
    warmup transposes to keep the clock ramped.
  - scores are computed TRANSPOSED: scT[k,q] = sum_e xT[e,k] yT[e,q],
    reusing yT as the moving operand (no kT).  The causal mask (also
    transposed) is added on DVE and exp (scale fused) runs on ACT in
    fp32r - no max-subtraction needed at fp32 range, and no PE
    transposes of the attention matrix at all.
  - softmax normalization is deferred: row sums come from a tiny
    ones-column matmul over the k partitions (psum [q,1]), 1/sum on
    DVE, and the scale is fused into the output evacuation as one DVE
    scalar_tensor_tensor: o = (po * rec) + resid -> fp16 store.
  - Z = X P^T token-major in fp16 psum->f32r tiles (ACT-copy evac);
    out = exp_T^T @ Z accumulated per (qt, eoh) with the causally-zero
    (qt0, ktt1) block skipped entirely.
  - queues: sync carries weights/xT/xres prefetches and eoh0 stores;
    gpsimd carries constants and eoh1 stores; ACT stays on the softmax
    critical path; output stores never queue behind prefetches.
"""
import sys

sys.path.insert(0, "/opt/trn_rl_repo")

import numpy as np

import concourse.bass as bass
import concourse.bacc as bacc
import concourse.mybir as mybir
import concourse.tile as tile
from concourse.bass_utils import run_bass_kernel_spmd

F32 = mybir.dt.float32
F32R = mybir.dt.float32r
F16 = mybir.dt.float16
AF = mybir.ActivationFunctionType

E = 1024          # embed dim
ET = E // 128     # e-tiles
W = 256           # window size
NW = 8            # windows per core
T = NW * W        # tokens per core
N_CORES = 8
SCALE = (E // 16) ** (-0.5)  # HEAD_DIM ** -0.5 = 0.125
NEG = -1.0e30

GROUPS = [(0,), (1,), (2, 3), (4, 5), (6, 7)]


def build_nc(nw=NW):
    t_core = nw * W
    nc = bacc.Bacc("TRN2", target_bir_lowering=False, debug=False)

    # x: token-major residual copy with b' = Wo@bv + bo pre-added (host)
    x_d = nc.dram_tensor("x", [t_core, E], F16, kind="ExternalInput")
    xt_d = nc.dram_tensor("xt", [E, t_core], F16, kind="ExternalInput")
    # windows 0/1 pre-packed on the host in SBUF tile layout -> 4KB DMA rows
    xs_d = [
        nc.dram_tensor(f"xs{i}", [128, ET * W], F16, kind="ExternalInput")
        for i in range(2)
    ]
    m_d = nc.dram_tensor("wm", [E, E], F16, kind="ExternalInput")   # Wq^T @ Wk
    p_d = nc.dram_tensor("wz", [E, E], F16, kind="ExternalInput")   # (Wo @ Wv)^T
    vv_d = nc.dram_tensor("vv", [128, ET], F32, kind="ExternalInput")  # Wk^T @ bq
    o_d = nc.dram_tensor("o", [t_core, E], F16, kind="ExternalOutput")

    # host-side constants baked into the NEFF
    # transposed-layout causal masks: maskT[ktt][k, q] = NEG where q < k
    # (scores are computed as scT[k, q]; softmax runs along the k partitions)
    mask_np = np.zeros((2, 128, W), dtype=np.float32)
    k_i = np.arange(128)[:, None]
    q_i = np.arange(W)[None, :]
    mask_np[0][q_i < k_i] = NEG
    mask_np[1][:, :128][q_i[:, :128] < k_i] = NEG
    mask_d = nc.inline_tensor(mask_np, "mask")
    ident_d = nc.inline_tensor(np.eye(128, dtype=np.float16), "ident")
    ones_d = nc.inline_tensor(np.ones((128, 4), dtype=np.float32), "ones")

    with tile.TileContext(nc) as tc:
        with (
            tc.tile_pool(name="wgt", bufs=1) as wp,
            tc.tile_pool(name="cp", bufs=1) as cp,
            tc.tile_pool(name="xp", bufs=4) as xp,
            tc.tile_pool(name="xtp", bufs=2) as xtp,
            tc.tile_pool(name="ytp", bufs=2) as ytp,
            tc.tile_pool(name="zp", bufs=2) as zp,
            tc.tile_pool(name="sp", bufs=2) as sp,
            tc.tile_pool(name="smp", bufs=4) as smp,
            tc.tile_pool(name="op", bufs=3) as op,
            tc.tile_pool(name="ps_y", bufs=2, space=bass.MemorySpace.PSUM) as ps_y,
            tc.tile_pool(name="ps_z", bufs=2, space=bass.MemorySpace.PSUM) as ps_z,
            tc.tile_pool(name="ps_o", bufs=3, space=bass.MemorySpace.PSUM) as ps_o,
            tc.tile_pool(name="ps_tr", bufs=1, space=bass.MemorySpace.PSUM) as ps_tr,
        ):
            # ---- resident constants (gpsimd queue; tiny) ----
            ident = cp.tile([128, 128], F16, tag="ident")
            nc.gpsimd.dma_start(ident[:], ident_d.ap())
            masks = cp.tile([128, 2, W], F32, tag="mask")
            for qt in range(2):
                nc.gpsimd.dma_start(masks[:, qt, :], mask_d.ap()[qt])
            vv_sb = cp.tile([128, ET], F32, tag="vv")
            nc.gpsimd.dma_start(vv_sb[:], vv_d.ap())
            ones_col = cp.tile([128, 4], F32R, tag="ones")
            nc.gpsimd.dma_start(ones_col[:], ones_d.ap().bitcast(F32R))

            # ---- resident weights: wsb[m][p, ei, eo] = Wm[ei*128+p, eo] ----
            wsb = {
                "m": wp.tile([128, ET, E], F16, tag="wm", name="wmsb"),
                "z": wp.tile([128, ET, E], F16, tag="wz", name="wzsb"),
            }
            w_d = {"m": m_d, "z": p_d}

            def load_weight(m, half, warm=False, eq_list=(0, 4)):
                # one 3D DMA per (half, ei-quadrant): 0.5MB transfers with
                # 1KB contiguous rows keep the DMA engines near full rate
                wr = w_d[m].ap().rearrange("(a p) n -> a p n", p=128)
                for eq in eq_list:
                    nc.sync.dma_start(
                        wsb[m][:, eq : eq + 4, half * 512 : (half + 1) * 512],
                        wr[eq : eq + 4, :, half * 512 : (half + 1) * 512].transpose(
                            [1, 0, 2]
                        ),
                    )
                    if warm:
                        # keep the PE activity monitor warm through the
                        # DMA-bound phase: a tiny transpose per arriving
                        # chunk, paced by the DMA itself
                        wps = ps_tr.tile([128, 128], F16, tag="tr", name="warm")
                        nc.tensor.transpose(
                            wps[:],
                            wsb[m][:, eq, half * 512 : half * 512 + 128],
                            ident[:],
                        )

            xtr = xt_d.ap().rearrange("(a p) t -> a p t", p=128)
            xT_tiles = {}

            def load_xT(gi, parts=(0, 1)):
                g = GROUPS[gi]
                if len(g) == 1:
                    # solo startup windows: host-packed tile layout loads as
                    # linear DMAs with 4KB rows; split by ei-half so the
                    # first y matmuls (ei 0-3) start one chunk earlier
                    if gi not in xT_tiles:
                        xT_tiles[gi] = xtp.tile([128, ET, W], F16, tag="xT0", name=f"xT0_{gi}")
                    t = xT_tiles[gi]
                    srcr = xs_d[g[0]].ap().rearrange("p (a t) -> p a t", a=ET)
                    for pt in parts:
                        nc.sync.dma_start(
                            t[:, pt * 4 : (pt + 1) * 4, :],
                            srcr[:, pt * 4 : (pt + 1) * 4, :],
                        )
                    return
                else:
                    t = xtp.tile([128, ET, 2 * W], F16, tag="xT")
                    nc.sync.dma_start(
                        t[:, :, :],
                        xtr[:, :, g[0] * W : (g[0] + 2) * W].transpose([1, 0, 2]),
                    )
                xT_tiles[gi] = t

            for gi, g in enumerate(GROUPS):
                pw = len(g) * W

                if gi == 0:
                    # sync-queue order = deadline order for window 0/1:
                    # M-h0 + xs0 gate the first y matmuls; M-h1 by y-eo4
                    # (~+3us); P halves by the w0 Z-projection (~+8/10us);
                    # xs1 only by w1's y (~+16us)
                    load_weight("m", half=0, warm=True, eq_list=(0,))
                    load_xT(0, parts=(0,))
                    load_weight("m", half=0, warm=True, eq_list=(4,))
                    load_xT(0, parts=(1,))
                    load_weight("m", half=1, warm=True)
                    load_weight("z", half=0, warm=True)
                    load_weight("z", half=1, warm=True)
                    load_xT(1)  # prefetch window 1 behind the weights
                elif gi + 1 < len(GROUPS):
                    load_xT(gi + 1)
                xT = xT_tiles[gi]

                # ---- y projection -> [e_out, t] layout, vvec bias fused ----
                yT = ytp.tile([128, ET, 2 * W], F16, tag="yT")
                for eo in range(ET):
                    pp = ps_y.tile([128, 2 * W], F32, tag="y")
                    for ei in range(ET):
                        nc.tensor.matmul(
                            pp[:, :pw],
                            wsb["m"][:, ei, eo * 128 : (eo + 1) * 128],
                            xT[:, ei, :pw],
                            start=(ei == 0),
                            stop=(ei == ET - 1),
                        )
                    if eo % 2 == 0:
                        nc.scalar.add(
                            yT[:, eo, :pw], pp[:, :pw], vv_sb[:, eo : eo + 1]
                        )
                    else:
                        nc.vector.tensor_scalar_add(
                            yT[:, eo, :pw], pp[:, :pw], vv_sb[:, eo : eo + 1]
                        )


                for wi, w in enumerate(g):
                    tok0 = w * W
                    two0 = wi * W  # token offset inside the group tiles

                    # residual (x + b'), token-major; needed only at out evac
                    x_w = []
                    for tt in range(2):
                        xt_ = xp.tile([128, E], F16, tag="x")
                        nc.sync.dma_start(
                            xt_[:],
                            x_d.ap()[tok0 + tt * 128 : tok0 + (tt + 1) * 128, :],
                        )
                        x_w.append(xt_)

                    # ---- transposed scores scT[k, q] + unnormalized exp ----
                    # softmax normalization is deferred: row sums come from a
                    # ones-column matmul over the k partitions and 1/sum is
                    # fused into the output evacuation.  exp stays fp32 so no
                    # max-subtraction is needed (f32 range covers e^30).
                    ex = []
                    for ktt in range(2):
                        qlo = ktt * 128  # causal: k-block ktt only sees q >= qlo
                        qw = W - qlo
                        sc = ps_z.tile([128, 512], F32, tag="z")
                        for ei in range(ET):
                            nc.tensor.matmul(
                                sc[:, :qw],
                                xT[:, ei, two0 + ktt * 128 : two0 + (ktt + 1) * 128],
                                yT[:, ei, two0 + qlo : two0 + W],
                                start=(ei == 0),
                                stop=(ei == ET - 1),
                            )
                        e_sb = sp.tile([128, W], F32R, tag="s", name=f"e{ktt}")
                        nc.vector.tensor_add(
                            e_sb[:, :qw], sc[:, :qw], masks[:, ktt, :qw]
                        )
                        nc.scalar.activation(
                            e_sb[:, :qw], e_sb[:, :qw], AF.Exp, scale=SCALE
                        )
                        ex.append(e_sb)


                    # ---- Z projection (token-major): Z = X P^T ----
                    z_w = [
                        zp.tile([128, E], F32R, tag="z", name=f"z{tt}")
                        for tt in range(2)
                    ]
                    for eoh in range(2):
                        for tt in range(2):
                            pv = ps_z.tile([128, 512], F32, tag="z")
                            for ei in range(ET):
                                nc.tensor.matmul(
                                    pv[:],
                                    xT[:, ei, two0 + tt * 128 : two0 + (tt + 1) * 128],
                                    wsb["z"][:, ei, eoh * 512 : (eoh + 1) * 512],
                                    start=(ei == 0),
                                    stop=(ei == ET - 1),
                                )
                            nc.scalar.copy(
                                z_w[tt][:, eoh * 512 : (eoh + 1) * 512], pv[:]
                            )

                    # ---- row sums (over k partitions) and reciprocals ----
                    recs = []
                    for qt in range(2):
                        # exp block for query column range qt: ex[0][:, qt*128:...]
                        # plus (qt1 only) ex[1][:, 0:128]
                        srcs = [(0, 0)] if qt == 0 else [(0, 128), (1, 0)]
                        sm = ps_o.tile([128, 512], F32, tag="o")
                        for i, (ktt, qo) in enumerate(srcs):
                            nc.tensor.matmul(
                                sm[:, :4],
                                ex[ktt][:, qo : qo + 128],
                                ones_col[:],
                                start=(i == 0),
                                stop=(i == len(srcs) - 1),
                            )
                        rec = smp.tile([128, 1], F32, tag="rec")
                        nc.vector.reciprocal(rec[:], sm[:, :1])
                        recs.append(rec)

                    # ---- out = (expT^T @ Z) * rec + residual, token-major ----
                    for qt in range(2):
                        srcs = [(0, 0)] if qt == 0 else [(0, 128), (1, 0)]
                        for eoh in range(2):
                            po = ps_o.tile([128, 512], F32, tag="o")
                            for i, (ktt, qo) in enumerate(srcs):
                                nc.tensor.matmul(
                                    po[:],
                                    ex[ktt][:, qo : qo + 128],
                                    z_w[ktt][:, eoh * 512 : (eoh + 1) * 512],
                                    start=(i == 0),
                                    stop=(i == len(srcs) - 1),
                                )
                            o_sb = op.tile([128, 512], F16, tag="o")
                            nc.vector.scalar_tensor_tensor(
                                o_sb[:],
                                po[:],
                                recs[qt][:],
                                x_w[qt][:, eoh * 512 : (eoh + 1) * 512],
                                op0=mybir.AluOpType.mult,
                                op1=mybir.AluOpType.add,
                            )
                            # stores on the sync/gpsimd queues (idle in
                            # steady state): the scalar queue's ACT ops sit
                            # on the softmax critical path, keep it clear
                            if w == nw - 1:
                                q = (nc.sync, nc.scalar, nc.gpsimd, nc.sync)[
                                    qt * 2 + eoh
                                ]
                            else:
                                q = nc.sync if eoh == 0 else nc.gpsimd
                            q.dma_start(
                                o_d.ap()[
                                    tok0 + qt * 128 : tok0 + (qt + 1) * 128,
                                    eoh * 512 : (eoh + 1) * 512,
                                ],
                                o_sb[:],
                                single_packet=(w == nw - 1),
                            )

    nc.compile()
    return nc


_NC_CACHE = {}


def _get_nc(nw=NW):
    if nw not in _NC_CACHE:
        _NC_CACHE[nw] = build_nc(nw)
    return _NC_CACHE[nw]


def _prep(x, Wq, bq, Wk, bk, Wv, bv, Wo, bo):
    """Host-side weight folding + per-core input maps."""
    x = np.asarray(x, dtype=np.float32)
    B, S, _ = x.shape
    Wq = np.asarray(Wq, np.float32)
    Wk = np.asarray(Wk, np.float32)
    Wv = np.asarray(Wv, np.float32)
    Wo = np.asarray(Wo, np.float32)
    bq = np.asarray(bq, np.float32)
    bv = np.asarray(bv, np.float32)
    bo = np.asarray(bo, np.float32)

    M = Wq.T @ Wk                      # scores = (X M) X^T  (+ col bias)
    Pt = (Wo @ Wv).T                   # Z = X @ Pt
    vvec = Wk.T @ bq                   # col bias, fused into y-projection
    bprime = Wo @ bv + bo              # folded into the residual below

    x_flat = x.reshape(B * S, E)
    x_resid = x_flat + bprime[None, :]
    t_core = B * S // N_CORES
    assert t_core == T

    common = {
        "wm": np.ascontiguousarray(M.astype(np.float16)),
        "wz": np.ascontiguousarray(Pt.astype(np.float16)),
        "vv": np.ascontiguousarray(vvec.reshape(ET, 128).T),
    }
    in_maps = []
    for i in range(N_CORES):
        xt_i = x_flat[i * t_core : (i + 1) * t_core].T.astype(np.float16)
        m = {
            "x": np.ascontiguousarray(
                x_resid[i * t_core : (i + 1) * t_core].astype(np.float16)
            ),
            "xt": np.ascontiguousarray(xt_i),
            **common,
        }
        for w in range(2):
            sl = xt_i[:, w * W : (w + 1) * W]  # [E, W]
            m[f"xs{w}"] = np.ascontiguousarray(
                sl.reshape(ET, 128, W).transpose(1, 0, 2).reshape(128, ET * W)
            )
        in_maps.append(m)
    return in_maps


def kernel(x, Wq, bq, Wk, bk, Wv, bv, Wo, bo):
    in_maps = _prep(x, Wq, bq, Wk, bk, Wv, bv, Wo, bo)
    B, S = np.asarray(x).shape[:2]
    nc = _get_nc()
    res = run_bass_kernel_spmd(nc, in_maps, core_ids=list(range(N_CORES)))
    out = np.concatenate([res.results[i]["o"] for i in range(N_CORES)], axis=0)
    return out.reshape(B, S, E).astype(np.float32)
